# revision 15
# baseline (speedup 1.0000x reference)
"""AffinityPropagate Trainium2 kernel.

Math (per batch image, reference semantics):
    w_k = |a_k| / sum_k |a_k|            (per-pixel, 9 taps, k=(dy,dx))
    f <- sum_k w_k * shift_k(pad0(f))    repeated 4 times

Sharding: pure data parallel - batch 8 -> 8 NeuronCores, one image each.

Layout per core (flat-chunk):
    The image is flattened to q = y*W + x in [0, H*W); partition p owns the
    contiguous pixel chunk [p*CH, (p+1)*CH), CH = H*W/128 = 4080.  The feature
    buffer [128, CH + 2*HA] stores each chunk with HA = W+1 halo pixels
    duplicated on both sides, so every 3x3 tap is a free-dim offset
    off = dy*W + dx.

    In flat indexing, a dx=-1 tap at x=0 wraps to the previous row's last
    pixel (and dx=+1 at x=W-1 to the next row's first), where the reference
    sees zero padding.  Since padding only zeroes the *feature* read (the
    denominator sum_k |a_k| still counts every tap), this is equivalent to
    zeroing those taps' weights at the wrap columns.

    Normalization is folded into the weights once: w = |a| * mask * (1/sum),
    computed chunk-by-chunk as the affinity stream arrives (one fused
    [128,3,3,cw] multiply against rm = mask3 * r16).  Iterations then need no
    per-pixel rescale - PSUM evacuation is a plain fp32->fp16 copy on the
    Activation engine.

    Engine split (steady-state iteration):
      DVE       ~7 tap-product planes per chunk as fused 4-dim fp16 muls (2x)
      Pool      2 tap-product planes per chunk (idle engine, slower rate)
      TensorE   9 wide identity matmuls per chunk accumulate planes in PSUM
      ScalarE   |a| converts (phase 1), PSUM evacuation copies, halo evac
      DMA       loads phase 1; halo partition-shift copies during iterations

    Halo refresh: phase-1 buffers get PE partition-shift matmuls (zero rows
    establish the outer zero padding); iteration buffers reuse those zeros and
    refresh the interior halo with partition-shifted SBUF->SBUF DMAs (the DMA
    queues are idle once the affinity stream finishes).

    Schedule: the 18.8MB fp32 affinity read is the serial HBM resource, so
    iteration 0 is cut into 1020-px chunks interleaved into the
    normalization stream as each weight range completes; iterations 1-3 run
    engine-balanced across DVE/Pool/PE with per-chunk pipelining.
"""

import numpy as np

import concourse.bacc as bacc
import concourse.bass as bass
import concourse.mybir as mybir
import concourse.tile as tile
from concourse.bass_utils import run_bass_kernel_spmd

H, W = 544, 960
NPIX = H * W
NK = 9
CH = NPIX // 128  # 4080 pixels per partition
HA = W + 1  # halo on each side
FW = CH + 2 * HA  # feature row length per partition
ITERS = 4
CW = 255  # norm column chunk (16 chunks)
CI = 1020  # iteration chunk (4 chunks)
MW = W + CW  # stored mask width (mask is W-periodic, reads start at c0 % W)
AF = mybir.AluOpType
DT = mybir.dt
F16 = DT.float16
F32 = DT.float32

# tap-product plane groups per iteration chunk: (engine, dy0, ndy, dx0, ndx)
# in index coords (0..2 ~ dy,dx = -1..+1).  Interior groups first; groups that
# read the chunk's halo side come last so the halo refresh can overlap.  The
# Pool engine gets 2 planes per chunk (1 on the last) to offload the DVE.
GROUPS = {
    0: [
        ("pool", 2, 1, 1, 2),
        ("vec", 1, 1, 1, 2),
        ("vec", 2, 1, 0, 1),
        ("vec", 0, 1, 0, 3),  # dy=-1 row: needs left halo
        ("vec", 1, 1, 0, 1),  # (0,-1): last col of left halo
    ],
    1: [
        ("pool", 2, 1, 1, 2),
        ("vec", 0, 2, 0, 3),
        ("vec", 2, 1, 0, 1),
    ],
    3: [
        ("pool", 0, 1, 0, 1),
        ("vec", 0, 1, 1, 2),
        ("vec", 1, 1, 0, 2),
        ("vec", 2, 1, 0, 3),  # dy=+1 row: needs right halo
        ("vec", 1, 1, 2, 1),  # (0,+1): first col of right halo
    ],
}
GROUPS[2] = GROUPS[1]
# matmul accumulation order per chunk (indices into GROUPS[c]): fast DVE
# products first, slow Pool plane mid, halo-dependent planes last.
MM_ORDER = {0: [1, 2, 0, 3, 4], 1: [1, 2, 0], 2: [1, 2, 0], 3: [1, 2, 0, 3, 4]}

_nc_cache = {}


def _build():
    nc = bacc.Bacc(
        "TRN2",
        target_bir_lowering=False,
        debug=False,
        enable_asserts=False,
    )
    a = nc.dram_tensor("a", [NK, H, W], F32, kind="ExternalInput").ap()
    f = nc.dram_tensor("f", [H, W], F32, kind="ExternalInput").ap()
    m = nc.dram_tensor("m", [128, 3, MW], F16, kind="ExternalInput").ap()
    ident = nc.dram_tensor("ident", [128, 3, 128], F16, kind="ExternalInput").ap()
    o = nc.dram_tensor("o", [H, W], F32, kind="ExternalOutput").ap()

    with tile.TileContext(nc) as tc:
        _build_tile(tc, a, f, m, ident, o)
    nc.finalize()
    return nc


def _bcast(sl, n):
    """Insert a [0, n] broadcast dim after the partition dim of an AP."""
    return bass.AP(
        tensor=sl.tensor, offset=sl.offset, ap=[sl.ap[0], [0, n], *sl.ap[1:]]
    )


def _build_tile(tc, a, f, m, ident, o):
    nc = tc.nc
    # flattened per-partition views of the DRAM tensors
    av = (
        a.rearrange("k h w -> k (h w)")
        .rearrange("k (p j) -> k p j", p=128)
        .rearrange("k p j -> p k j")
    )
    ff = f.rearrange("h w -> (h w)").rearrange("(p j) -> p j", p=128)
    of = o.rearrange("h w -> (h w)").rearrange("(p j) -> p j", p=128)

    with (
        tc.tile_pool(name="persist", bufs=1) as persist,
        tc.tile_pool(name="stage", bufs=2) as stage_pool,
        tc.tile_pool(name="small", bufs=2) as small,
        tc.tile_pool(name="prodp", bufs=2) as prodp,
        tc.tile_pool(name="outp", bufs=3) as outp,
        tc.tile_pool(name="psum", bufs=2, space="PSUM") as psump,
    ):
        fb = [persist.tile([128, FW], F16, name=f"f{i}") for i in range(2)]
        aw = persist.tile([128, NK, CH], F16, name="aw")
        msk3 = persist.tile([128, 3, MW], F16, name="msk3")
        idt3 = persist.tile([128, 3, 128], F16, name="idt3")

        idt = idt3[:, 0, :]
        sdn = idt3[:, 1, :]
        sup = idt3[:, 2, :]

        def aw4(dy0, ndy, dx0, ndx, c0, cw):
            return aw[:].rearrange("p (dy dx) c -> p dy dx c", dy=3)[
                :, dy0 : dy0 + ndy, dx0 : dx0 + ndx, c0 : c0 + cw
            ]

        def fview(ft, base, dy0, ndy, dx0, ndx, cw):
            """[128, ndy, ndx, cw] view of ft at tap offsets dy*W + dx."""
            sl = ft[:, 0:cw]
            return bass.AP(
                tensor=sl.tensor,
                offset=sl.offset + base + (dy0 - 1) * W + (dx0 - 1),
                ap=[sl.ap[0], [W, ndy], [1, ndx], *sl.ap[1:]],
            )

        def norm_chunk(ci, c0):
            st = stage_pool.tile([128, NK, CW], F32, name="st", tag="st", bufs=3)
            dmae = nc.sync if ci % 2 == 0 else nc.scalar
            dmae.dma_start(out=st[:], in_=av[:, :, c0 : c0 + CW])
            awc = aw[:, :, c0 : c0 + CW]
            nc.scalar.activation(
                out=awc, in_=st[:], func=mybir.ActivationFunctionType.Abs
            )
            s = psump.tile([128, CW], F32, name="s", tag="s")
            for k in range(NK):
                nc.tensor.matmul(
                    s[:],
                    idt[:],
                    aw[:, k, c0 : c0 + CW],
                    start=(k == 0),
                    stop=(k == NK - 1),
                )
            # 1/sum straight to fp16 (fast-recip spec, fp16 output AP)
            from concourse.dve_ops import (
                RECIP_APPROX_FAST_CONSTS,
                RECIPROCAL_APPROX_FAST,
            )

            r16 = small.tile([128, CW], F16, name="r16", tag="r16")
            c = RECIP_APPROX_FAST_CONSTS
            nc.vector._custom_dve(
                RECIPROCAL_APPROX_FAST,
                out=r16[:],
                in0=s[:],
                s0=c["s0"],
                s1=c["s1"],
                imm2=c["imm2"],
            )
            # rm rows (dx=-1,0,+1) = mask row * r; middle mask row is ones
            rm = small.tile([128, 3, CW], F16, name="rm", tag="rm")
            q0 = c0 % W
            nc.vector.tensor_mul(
                out=rm[:], in0=msk3[:, :, q0 : q0 + CW], in1=_bcast(r16[:], 3)
            )
            # fold mask * 1/sum into all 9 weight planes in one op
            awv = aw4(0, 3, 0, 3, c0, CW)
            rsl = rm[:]
            rmv = bass.AP(
                tensor=rsl.tensor, offset=rsl.offset, ap=[rsl.ap[0], [0, 3], *rsl.ap[1:]]
            )
            nc.vector.tensor_mul(out=awv, in0=awv, in1=rmv)

        acc0 = {}  # iter-0 PSUM tiles, evacuated by a later evac0() call

        def evac0(c):
            # deferred iter-0 evacuation: emitted a few norm chunks after the
            # products so the in-order Act engine doesn't stall abs converts
            # behind it while the matmuls are still in flight
            nc.scalar.copy(out=fb[1][:, HA + c * CI : HA + (c + 1) * CI], in_=acc0.pop(c)[:])

        def iter_chunk(t, c):
            fc, fn = fb[t % 2], fb[(t + 1) % 2]
            last = t == ITERS - 1
            c0 = c * CI
            base = HA + c0
            acc = psump.tile([128, CI], F32, name="acc", tag="acc")
            if t == 0:
                acc0[c] = acc
            prod = prodp.tile([128, 3, 3, CI], F16, name="prod", tag="prod")
            groups = GROUPS[c]
            for eng, dy0, ndy, dx0, ndx in groups:
                out = prod[:, dy0 : dy0 + ndy, dx0 : dx0 + ndx, :]
                in0 = aw4(dy0, ndy, dx0, ndx, c0, CI)
                in1 = fview(fc, base, dy0, ndy, dx0, ndx, CI)
                e = nc.gpsimd if eng == "pool" else nc.vector
                e.tensor_mul(out=out, in0=in0, in1=in1)
            planes = []
            for gi in MM_ORDER[c]:
                _, dy0, ndy, dx0, ndx = groups[gi]
                planes += [
                    (dy, dx)
                    for dy in range(dy0, dy0 + ndy)
                    for dx in range(dx0, dx0 + ndx)
                ]
            for pi, (dy, dx) in enumerate(planes):
                for s0 in range(0, CI, 512):  # matmul free size capped at 512
                    se = min(s0 + 512, CI)
                    nc.tensor.matmul(
                        acc[:, s0:se],
                        idt[:],
                        prod[:, dy, dx, s0:se],
                        start=(pi == 0),
                        stop=(pi == NK - 1),
                    )
            if last:
                for q0 in range(0, CI, 510):
                    ost = outp.tile([128, 510], F32, name="ost", tag="ost")
                    nc.scalar.copy(out=ost[:], in_=acc[:, q0 : q0 + 510])
                    nc.sync.dma_start(
                        out=of[:, c0 + q0 : c0 + q0 + 510], in_=ost[:]
                    )
            elif t > 0:
                # PSUM evacuation: plain copy (weights carry 1/sum already)
                nc.scalar.copy(out=fn[:, base : base + CI], in_=acc[:])
                # interior halo refresh by partition-shifted SBUF->SBUF DMA
                # (outermost halos stay at the zeros set by the PE refresh);
                # in phase 1 the DMA queues are busy with the affinity stream,
                # so iter 0 uses the PE refresh instead (emitted by caller).
                if t > 0 and c == 0:
                    nc.scalar.dma_start(
                        out=fn[0:127, HA + CH : FW], in_=fn[1:128, HA : 2 * HA]
                    )
                if t > 0 and c == 3:
                    nc.scalar.dma_start(
                        out=fn[1:128, 0:HA], in_=fn[0:127, CH : CH + HA]
                    )

        def refresh_pe(ft):
            """Full halo build on TensorE + ScalarE (phase 1 only).  The
            shift matrices' zero rows set the outermost halos to exactly 0 =
            the reference's dy zero padding."""
            phR = psump.tile([128, HA], F32, name="phR", tag="halo", bufs=1)
            for s0 in range(0, HA, 512):
                se = min(s0 + 512, HA)
                nc.tensor.matmul(
                    phR[:, s0:se], sup, ft[:, HA + s0 : HA + se], start=True, stop=True
                )
            nc.scalar.copy(out=ft[:, HA + CH : FW], in_=phR[:])
            phL = psump.tile([128, HA], F32, name="phL", tag="halo", bufs=1)
            for s0 in range(0, HA, 512):
                se = min(s0 + 512, HA)
                nc.tensor.matmul(
                    phL[:, s0:se], sdn, ft[:, CH + s0 : CH + se], start=True, stop=True
                )
            nc.scalar.copy(out=ft[:, 0:HA], in_=phL[:])

        # ---- schedule ----
        nc.sync.dma_start(out=idt3[:], in_=ident)
        nc.scalar.dma_start(out=msk3[:], in_=m)
        for ci in range(4):
            norm_chunk(ci, ci * CW)
            if ci < 2:  # feature load + fp32->fp16 convert on the idle Pool
                fst = stage_pool.tile([128, 2040], F32, name="fst", tag="fst")
                nc.sync.dma_start(out=fst[:], in_=ff[:, ci * 2040 : (ci + 1) * 2040])
                nc.gpsimd.tensor_copy(
                    out=fb[0][:, HA + ci * 2040 : HA + (ci + 1) * 2040], in_=fst[:]
                )
        refresh_pe(fb[0])
        iter_chunk(0, 0)
        for ci in range(4, 16):
            norm_chunk(ci, ci * CW)
            if ci % 4 == 3:
                iter_chunk(0, ci // 4)
            if ci % 4 == 1 and ci > 4:
                evac0((ci - 5) // 4)
        evac0(3)
        refresh_pe(fb[1])

        for t in range(1, ITERS):
            for c in range(4):
                iter_chunk(t, c)


def _masks():
    # msk3[p, row, col]: wrap-column masks at x = (240*(p%4) + col) mod W -
    # partition p starts at pixel 4080p and 4080 mod W = 240, so the
    # W-periodic masks have 4 partition phases.  Rows are dx = -1 (zero at
    # x==0), dx = 0 (ones), dx = +1 (zero at x==W-1).
    col = np.arange(MW)
    out = np.empty((128, 3, MW), np.float16)
    for ph in range(4):
        x = (240 * ph + col) % W
        out[ph::4, 0] = (x != 0).astype(np.float16)
        out[ph::4, 1] = 1.0
        out[ph::4, 2] = (x != W - 1).astype(np.float16)
    return out


def _get_nc():
    if "nc" not in _nc_cache:
        _nc_cache["nc"] = _build()
    return _nc_cache["nc"]


def _run(affinity, feature, **spmd_kwargs):
    affinity = np.ascontiguousarray(np.asarray(affinity, dtype=np.float32))
    feature = np.ascontiguousarray(np.asarray(feature, dtype=np.float32))
    nbatch = affinity.shape[0]
    nc = _get_nc()
    masks = _masks()
    ident = np.ascontiguousarray(
        np.stack(
            [
                np.eye(128, dtype=np.float16),
                np.eye(128, k=1, dtype=np.float16),
                np.eye(128, k=-1, dtype=np.float16),
            ]
        ).transpose(1, 0, 2)
    )
    in_maps = [
        {"a": affinity[i], "f": feature[i, 0], "m": masks, "ident": ident}
        for i in range(nbatch)
    ]
    res = run_bass_kernel_spmd(nc, in_maps, core_ids=list(range(nbatch)), **spmd_kwargs)
    out = np.stack([r["o"] for r in res.results])[:, None, :, :]
    return out.astype(np.float32), res


def kernel(affinity, feature):
    out, _ = _run(affinity, feature)
    return out


# revision 18
# speedup vs baseline: 1.1031x; 1.1031x over previous
"""AffinityPropagate Trainium2 kernel.

Math (per batch image, reference semantics):
    w_k = |a_k| / sum_k |a_k|            (per-pixel, 9 taps, k=(dy,dx))
    f <- sum_k w_k * shift_k(pad0(f))    repeated 4 times

Sharding: pure data parallel - batch 8 -> 8 NeuronCores, one image each.

Layout per core (flat-chunk):
    The image is flattened to q = y*W + x in [0, H*W); partition p owns the
    contiguous pixel chunk [p*CH, (p+1)*CH), CH = H*W/128 = 4080.  The feature
    buffer [128, CH + 2*HA] stores each chunk with HA = W+1 halo pixels
    duplicated on both sides, so every 3x3 tap is a free-dim offset
    off = dy*W + dx.

    In flat indexing, a dx=-1 tap at x=0 wraps to the previous row's last
    pixel (and dx=+1 at x=W-1 to the next row's first), where the reference
    sees zero padding.  Since padding only zeroes the *feature* read (the
    denominator sum_k |a_k| still counts every tap), this is equivalent to
    zeroing those taps' weights at the wrap columns.

    Normalization is folded into the weights once: w = |a| * mask * (1/sum),
    computed chunk-by-chunk as the affinity stream arrives (one fused
    [128,3,3,cw] multiply against rm = mask3 * r16).  Iterations then need no
    per-pixel rescale - PSUM evacuation is a plain fp32->fp16 copy on the
    Activation engine.

    Engine split (steady-state iteration):
      DVE       ~7 tap-product planes per chunk as fused 4-dim fp16 muls (2x)
      Pool      2 tap-product planes per chunk (idle engine, slower rate)
      TensorE   9 wide identity matmuls per chunk accumulate planes in PSUM
      ScalarE   |a| converts (phase 1), PSUM evacuation copies, halo evac
      DMA       loads phase 1; halo partition-shift copies during iterations

    Halo refresh: phase-1 buffers get PE partition-shift matmuls (zero rows
    establish the outer zero padding); iteration buffers reuse those zeros and
    refresh the interior halo with partition-shifted SBUF->SBUF DMAs (the DMA
    queues are idle once the affinity stream finishes).

    Schedule: the 18.8MB fp32 affinity read is the serial HBM resource, so
    iteration 0 is cut into 1020-px chunks interleaved into the
    normalization stream as each weight range completes; iterations 1-3 run
    engine-balanced across DVE/Pool/PE with per-chunk pipelining.
"""

import numpy as np

import concourse.bacc as bacc
import concourse.bass as bass
import concourse.mybir as mybir
import concourse.tile as tile
from concourse.bass_utils import run_bass_kernel_spmd

H, W = 544, 960
NPIX = H * W
NK = 9
CH = NPIX // 128  # 4080 pixels per partition
HA = W + 1  # halo on each side
FW = CH + 2 * HA  # feature row length per partition
ITERS = 4
CW = 255  # norm column chunk (16 chunks)
CI = 1020  # iteration chunk (4 chunks)
MW = W + CW  # stored mask width (mask is W-periodic, reads start at c0 % W)
NFILL = 24  # phase-1 PE warm-keeping filler matmuls per norm chunk
AF = mybir.AluOpType
DT = mybir.dt
F16 = DT.float16
F32 = DT.float32

# tap-product plane groups per iteration chunk: (engine, dy0, ndy, dx0, ndx)
# in index coords (0..2 ~ dy,dx = -1..+1).  Interior groups first; groups that
# read the chunk's halo side come last so the halo refresh can overlap.  The
# Pool engine gets 2 planes per chunk (1 on the last) to offload the DVE.
GROUPS = {
    0: [
        ("pool", 2, 1, 1, 2),
        ("vec", 1, 1, 1, 2),
        ("vec", 2, 1, 0, 1),
        ("vec", 0, 1, 0, 3),  # dy=-1 row: needs left halo
        ("vec", 1, 1, 0, 1),  # (0,-1): last col of left halo
    ],
    1: [
        ("pool", 2, 1, 1, 2),
        ("vec", 0, 2, 0, 3),
        ("vec", 2, 1, 0, 1),
    ],
    3: [
        ("pool", 0, 1, 0, 1),
        ("vec", 0, 1, 1, 2),
        ("vec", 1, 1, 0, 2),
        ("vec", 2, 1, 0, 3),  # dy=+1 row: needs right halo
        ("vec", 1, 1, 2, 1),  # (0,+1): first col of right halo
    ],
}
GROUPS[2] = GROUPS[1]
# matmul accumulation order per chunk (indices into GROUPS[c]): fast DVE
# products first, slow Pool plane mid, halo-dependent planes last.
MM_ORDER = {0: [1, 2, 0, 3, 4], 1: [1, 2, 0], 2: [1, 2, 0], 3: [1, 2, 0, 3, 4]}

_nc_cache = {}


def _build():
    nc = bacc.Bacc(
        "TRN2",
        target_bir_lowering=False,
        debug=False,
        enable_asserts=False,
    )
    a = nc.dram_tensor("a", [NK, H, W], F32, kind="ExternalInput").ap()
    f = nc.dram_tensor("f", [H, W], F32, kind="ExternalInput").ap()
    m = nc.dram_tensor("m", [128, 3, MW], F16, kind="ExternalInput").ap()
    ident = nc.dram_tensor("ident", [128, 3, 128], F16, kind="ExternalInput").ap()
    o = nc.dram_tensor("o", [H, W], F32, kind="ExternalOutput").ap()

    with tile.TileContext(nc) as tc:
        _build_tile(tc, a, f, m, ident, o)
    nc.finalize()
    return nc


def _bcast(sl, n):
    """Insert a [0, n] broadcast dim after the partition dim of an AP."""
    return bass.AP(
        tensor=sl.tensor, offset=sl.offset, ap=[sl.ap[0], [0, n], *sl.ap[1:]]
    )


def _build_tile(tc, a, f, m, ident, o):
    nc = tc.nc
    # flattened per-partition views of the DRAM tensors
    av = (
        a.rearrange("k h w -> k (h w)")
        .rearrange("k (p j) -> k p j", p=128)
        .rearrange("k p j -> p k j")
    )
    ff = f.rearrange("h w -> (h w)").rearrange("(p j) -> p j", p=128)
    of = o.rearrange("h w -> (h w)").rearrange("(p j) -> p j", p=128)

    with (
        tc.tile_pool(name="persist", bufs=1) as persist,
        tc.tile_pool(name="stage", bufs=2) as stage_pool,
        tc.tile_pool(name="small", bufs=2) as small,
        tc.tile_pool(name="prodp", bufs=2) as prodp,
        tc.tile_pool(name="outp", bufs=3) as outp,
        tc.tile_pool(name="psum", bufs=2, space="PSUM") as psump,
    ):
        fb = [persist.tile([128, FW], F16, name=f"f{i}") for i in range(2)]
        aw = persist.tile([128, NK, CH], F16, name="aw")
        msk3 = persist.tile([128, 3, MW], F16, name="msk3")
        idt3 = persist.tile([128, 3, 128], F16, name="idt3")

        idt = idt3[:, 0, :]
        sdn = idt3[:, 1, :]
        sup = idt3[:, 2, :]

        def aw4(dy0, ndy, dx0, ndx, c0, cw):
            return aw[:].rearrange("p (dy dx) c -> p dy dx c", dy=3)[
                :, dy0 : dy0 + ndy, dx0 : dx0 + ndx, c0 : c0 + cw
            ]

        def fview(ft, base, dy0, ndy, dx0, ndx, cw):
            """[128, ndy, ndx, cw] view of ft at tap offsets dy*W + dx."""
            sl = ft[:, 0:cw]
            return bass.AP(
                tensor=sl.tensor,
                offset=sl.offset + base + (dy0 - 1) * W + (dx0 - 1),
                ap=[sl.ap[0], [W, ndy], [1, ndx], *sl.ap[1:]],
            )

        def norm_chunk(ci, c0):
            st = stage_pool.tile([128, NK, CW], F32, name="st", tag="st", bufs=3)
            dmae = nc.sync if ci % 2 == 0 else nc.scalar
            dmae.dma_start(out=st[:], in_=av[:, :, c0 : c0 + CW])
            awc = aw[:, :, c0 : c0 + CW]
            nc.scalar.activation(
                out=awc, in_=st[:], func=mybir.ActivationFunctionType.Abs
            )
            s = psump.tile([128, CW], F32, name="s", tag="s")
            # Always-ready 128-col filler matmuls keep the PE continuously
            # busy under the DMA-paced phase-1 stream: the cost model prices
            # a matmul by how long the PE has been busy without a gap
            # (p-state ramp), and the real iter-0 bursts otherwise launch
            # from idle at the 0.65 GHz p-state, 3.7x slower.  The fillers
            # write scratch that the first start=True sum matmul overwrites.
            for _ in range(NFILL):
                nc.tensor.matmul(s[:, 0:128], idt[:], idt[:], start=True, stop=True)
            for k in range(NK):
                nc.tensor.matmul(
                    s[:],
                    idt[:],
                    aw[:, k, c0 : c0 + CW],
                    start=(k == 0),
                    stop=(k == NK - 1),
                )
            # 1/sum straight to fp16 (fast-recip spec, fp16 output AP)
            from concourse.dve_ops import (
                RECIP_APPROX_FAST_CONSTS,
                RECIPROCAL_APPROX_FAST,
            )

            r16 = small.tile([128, CW], F16, name="r16", tag="r16")
            c = RECIP_APPROX_FAST_CONSTS
            nc.vector._custom_dve(
                RECIPROCAL_APPROX_FAST,
                out=r16[:],
                in0=s[:],
                s0=c["s0"],
                s1=c["s1"],
                imm2=c["imm2"],
            )
            # rm rows (dx=-1,0,+1) = mask row * r; middle mask row is ones
            rm = small.tile([128, 3, CW], F16, name="rm", tag="rm")
            q0 = c0 % W
            nc.vector.tensor_mul(
                out=rm[:], in0=msk3[:, :, q0 : q0 + CW], in1=_bcast(r16[:], 3)
            )
            # fold mask * 1/sum into all 9 weight planes in one op
            awv = aw4(0, 3, 0, 3, c0, CW)
            rsl = rm[:]
            rmv = bass.AP(
                tensor=rsl.tensor, offset=rsl.offset, ap=[rsl.ap[0], [0, 3], *rsl.ap[1:]]
            )
            nc.vector.tensor_mul(out=awv, in0=awv, in1=rmv)

        acc0 = {}  # iter-0 PSUM tiles, evacuated by a later evac0() call

        def evac0(c):
            # deferred iter-0 evacuation: emitted a few norm chunks after the
            # products so the in-order Act engine doesn't stall abs converts
            # behind it while the matmuls are still in flight
            nc.scalar.copy(out=fb[1][:, HA + c * CI : HA + (c + 1) * CI], in_=acc0.pop(c)[:])

        def iter_chunk(t, c):
            fc, fn = fb[t % 2], fb[(t + 1) % 2]
            last = t == ITERS - 1
            c0 = c * CI
            base = HA + c0
            acc = psump.tile([128, CI], F32, name="acc", tag="acc")
            if t == 0:
                acc0[c] = acc
            prod = prodp.tile([128, 3, 3, CI], F16, name="prod", tag="prod")
            groups = GROUPS[c]
            for eng, dy0, ndy, dx0, ndx in groups:
                out = prod[:, dy0 : dy0 + ndy, dx0 : dx0 + ndx, :]
                in0 = aw4(dy0, ndy, dx0, ndx, c0, CI)
                in1 = fview(fc, base, dy0, ndy, dx0, ndx, CI)
                e = nc.gpsimd if eng == "pool" else nc.vector
                e.tensor_mul(out=out, in0=in0, in1=in1)
            planes = []
            for gi in MM_ORDER[c]:
                _, dy0, ndy, dx0, ndx = groups[gi]
                planes += [
                    (dy, dx)
                    for dy in range(dy0, dy0 + ndy)
                    for dx in range(dx0, dx0 + ndx)
                ]
            for pi, (dy, dx) in enumerate(planes):
                for s0 in range(0, CI, 512):  # matmul free size capped at 512
                    se = min(s0 + 512, CI)
                    nc.tensor.matmul(
                        acc[:, s0:se],
                        idt[:],
                        prod[:, dy, dx, s0:se],
                        start=(pi == 0),
                        stop=(pi == NK - 1),
                    )
            if last:
                for q0 in range(0, CI, 510):
                    ost = outp.tile([128, 510], F32, name="ost", tag="ost")
                    nc.scalar.copy(out=ost[:], in_=acc[:, q0 : q0 + 510])
                    nc.sync.dma_start(
                        out=of[:, c0 + q0 : c0 + q0 + 510], in_=ost[:]
                    )
            elif t > 0:
                # PSUM evacuation: plain copy (weights carry 1/sum already)
                nc.scalar.copy(out=fn[:, base : base + CI], in_=acc[:])
                # interior halo refresh by partition-shifted SBUF->SBUF DMA
                # (outermost halos stay at the zeros set by the PE refresh);
                # in phase 1 the DMA queues are busy with the affinity stream,
                # so iter 0 uses the PE refresh instead (emitted by caller).
                if t > 0 and c == 0:
                    nc.scalar.dma_start(
                        out=fn[0:127, HA + CH : FW], in_=fn[1:128, HA : 2 * HA]
                    )
                if t > 0 and c == 3:
                    nc.scalar.dma_start(
                        out=fn[1:128, 0:HA], in_=fn[0:127, CH : CH + HA]
                    )

        def refresh_pe(ft):
            """Full halo build on TensorE + ScalarE (phase 1 only).  The
            shift matrices' zero rows set the outermost halos to exactly 0 =
            the reference's dy zero padding."""
            phR = psump.tile([128, HA], F32, name="phR", tag="halo", bufs=1)
            for s0 in range(0, HA, 512):
                se = min(s0 + 512, HA)
                nc.tensor.matmul(
                    phR[:, s0:se], sup, ft[:, HA + s0 : HA + se], start=True, stop=True
                )
            nc.scalar.copy(out=ft[:, HA + CH : FW], in_=phR[:])
            phL = psump.tile([128, HA], F32, name="phL", tag="halo", bufs=1)
            for s0 in range(0, HA, 512):
                se = min(s0 + 512, HA)
                nc.tensor.matmul(
                    phL[:, s0:se], sdn, ft[:, CH + s0 : CH + se], start=True, stop=True
                )
            nc.scalar.copy(out=ft[:, 0:HA], in_=phL[:])

        # ---- schedule ----
        nc.sync.dma_start(out=idt3[:], in_=ident)
        nc.scalar.dma_start(out=msk3[:], in_=m)
        for ci in range(4):
            norm_chunk(ci, ci * CW)
            if ci < 2:  # feature load + fp32->fp16 convert on the idle Pool
                fst = stage_pool.tile([128, 2040], F32, name="fst", tag="fst")
                nc.sync.dma_start(out=fst[:], in_=ff[:, ci * 2040 : (ci + 1) * 2040])
                nc.gpsimd.tensor_copy(
                    out=fb[0][:, HA + ci * 2040 : HA + (ci + 1) * 2040], in_=fst[:]
                )
        refresh_pe(fb[0])
        iter_chunk(0, 0)
        for ci in range(4, 16):
            norm_chunk(ci, ci * CW)
            if ci % 4 == 3:
                iter_chunk(0, ci // 4)
            if ci % 4 == 3 and ci >= 7:
                evac0((ci - 7) // 4)
        evac0(3)
        refresh_pe(fb[1])

        for t in range(1, ITERS):
            for c in range(4):
                iter_chunk(t, c)


def _masks():
    # msk3[p, row, col]: wrap-column masks at x = (240*(p%4) + col) mod W -
    # partition p starts at pixel 4080p and 4080 mod W = 240, so the
    # W-periodic masks have 4 partition phases.  Rows are dx = -1 (zero at
    # x==0), dx = 0 (ones), dx = +1 (zero at x==W-1).
    col = np.arange(MW)
    out = np.empty((128, 3, MW), np.float16)
    for ph in range(4):
        x = (240 * ph + col) % W
        out[ph::4, 0] = (x != 0).astype(np.float16)
        out[ph::4, 1] = 1.0
        out[ph::4, 2] = (x != W - 1).astype(np.float16)
    return out


def _get_nc():
    if "nc" not in _nc_cache:
        _nc_cache["nc"] = _build()
    return _nc_cache["nc"]


def _run(affinity, feature, **spmd_kwargs):
    affinity = np.ascontiguousarray(np.asarray(affinity, dtype=np.float32))
    feature = np.ascontiguousarray(np.asarray(feature, dtype=np.float32))
    nbatch = affinity.shape[0]
    nc = _get_nc()
    masks = _masks()
    ident = np.ascontiguousarray(
        np.stack(
            [
                np.eye(128, dtype=np.float16),
                np.eye(128, k=1, dtype=np.float16),
                np.eye(128, k=-1, dtype=np.float16),
            ]
        ).transpose(1, 0, 2)
    )
    in_maps = [
        {"a": affinity[i], "f": feature[i, 0], "m": masks, "ident": ident}
        for i in range(nbatch)
    ]
    res = run_bass_kernel_spmd(nc, in_maps, core_ids=list(range(nbatch)), **spmd_kwargs)
    out = np.stack([r["o"] for r in res.results])[:, None, :, :]
    return out.astype(np.float32), res


def kernel(affinity, feature):
    out, _ = _run(affinity, feature)
    return out
